# revision 58
# baseline (speedup 1.0000x reference)
"""Causal multi-head attention block (B=2, T=2048, C=1024, H=16) on 8 TRN2 cores.

Sharding: tensor-parallel over heads x data-parallel over batch.
Core c handles batch b = c // 4 and head-group hg = c % 4 (4 heads = 256 of
the 1024 channel columns). Each core computes, for its batch and heads:
    QT/KT = (Wqk/8^0.5-ish)^T X^T + b  (fp8e4m3 DoubleRow matmuls; host sends
            fp8 copies of X^T and 64x-scaled Wq|Wk, rescaled 2^-6 in the
            fused bias step; 1/sqrt(D) split over Q and K for fp8 range)
    V     = X Wv + bv            (bf16 matmuls, bf16 V + ones column)
    S^T   = K Q^T per 128-key chunk (fp8 DoubleRow, zero-padded 2nd k-tile)
    P^T   = exp(S^T) -> bf16; causal mask applied post-exp as a
            multiplicative 0/1 [128,128] block on the Pool engine
    O     = P V per 128-query chunk (bf16): out[q,0:64]=sum(P*V),
            out[q,64]=sum(P); per-partition softmax normalize
    O^T   via PE transpose (identity matmul) of the [q, 2*64] head-pair block
    partial = O^T rows @ Wo_rows_slice -> OUT bf16 [2048, 1024]
Host sums the 4 partials per batch and adds bo.

Schedule notes (engines execute their streams IN ORDER; emission = schedule):
 - all input DMAs ride the SP queue (ACT-queue DMAs would serialize ahead of
   the exps in the ACT sequencer stream)
 - each head's AV+normalize block is deferred past the next head's first two
   score pairs so ACT streams exps continuously
 - stage-A/C units are interleaved as PE fillers inside the attention streams
 - matmul PSUM outputs must be bank-aligned; transposes need their own
   PSUM tag (sharing the po slot corrupts under rotation)
"""

from contextlib import ExitStack

import numpy as np

import concourse.bacc as bacc
import concourse.mybir as mybir
import concourse.tile as tile
from concourse.bass_utils import run_bass_kernel_spmd

B, T, C, H, D = 2, 2048, 1024, 16, 64
N_CORES = 8
HG = 4                  # head-groups (tensor parallel)
HPC = H // HG           # heads per core = 4
HD = HPC * D            # channel slice per core = 256
P = 128                 # partitions
NT = T // 512           # 4 i-tiles of 512
NIC = T // P            # 16 i-chunks of 128
NKC = C // P            # 8 contraction chunks of 128
F32 = mybir.dt.float32
FP8 = mybir.dt.float8e4
DR = mybir.MatmulPerfMode.DoubleRow
AF = mybir.ActivationFunctionType

MM_DT = mybir.dt.bfloat16   # projection/output matmul dtype
N_EX = 26                   # exp-pair buffers (cross-tile AV deferral liveness)
DEFER_AV = True             # emit each head's AV block after next head's scores

_CACHE: dict = {}


def _build_program():
    import ml_dtypes

    bf16 = ml_dtypes.bfloat16
    nc = bacc.Bacc("TRN2", debug=False)

    XT = nc.dram_tensor("XT", [C, T], MM_DT, kind="ExternalInput").ap()
    XT8 = nc.dram_tensor("XT8", [C, T], FP8, kind="ExternalInput").ap()
    WQK8 = nc.dram_tensor("WQK8", [C, 2 * HD], FP8, kind="ExternalInput").ap()
    WV = nc.dram_tensor("WV", [C, HD], MM_DT, kind="ExternalInput").ap()
    BQK = nc.dram_tensor("BQK", [P, 4], F32, kind="ExternalInput").ap()
    BV = nc.dram_tensor("BV", [1, HD], MM_DT, kind="ExternalInput").ap()
    WO = nc.dram_tensor("WO", [HD, C], MM_DT, kind="ExternalInput").ap()
    OUT = nc.dram_tensor("OUT", [T, C], MM_DT, kind="ExternalOutput").ap()


    # Multiplicative causal mask for the diagonal 128x128 block of each
    # diagonal key-chunk: element (p, j) valid iff j >= p.
    m01 = (np.arange(128)[None, :] >= np.arange(128)[:, None]).astype(bf16)
    M01 = nc.inline_tensor(m01, name="m01").ap()
    IDN = nc.inline_tensor(np.eye(128, dtype=bf16), name="idn").ap()
    ONES = nc.inline_tensor(np.ones((1, P), bf16), name="ones_c").ap()
    VONES = nc.inline_tensor(np.ones((P, NIC * HPC), bf16), name="vones_c").ap()

    with tile.TileContext(nc) as tc:
        _trace_kernel(tc, XT, XT8, WQK8, WV, BQK, BV, WO, OUT, M01, IDN, ONES, VONES)
    nc.compile()
    return nc


def _trace_kernel(tc, XT, XT8, WQK8, WV, BQK, BV, WO, OUT, M01, IDN, ONES, VONES):
    nc = tc.nc

    with ExitStack() as ctx:
        consts = ctx.enter_context(tc.tile_pool(name="consts", bufs=1))
        wpool = ctx.enter_context(tc.tile_pool(name="weights", bufs=1))
        xpool = ctx.enter_context(tc.tile_pool(name="xt", bufs=1))
        qkv = ctx.enter_context(tc.tile_pool(name="qkv", bufs=1))

        # Two HWDGE queues: SP (nc.sync) and Activation (nc.scalar).
        qs, qa = nc.sync, nc.scalar

        # ---- tiles ----
        m01_sb = consts.tile([P, P], MM_DT, name="m01_sb")
        idn_sb = consts.tile([P, P], MM_DT, name="idn_sb")
        ones_sb = consts.tile([1, P], MM_DT, name="ones_sb")
        bias_sb = consts.tile([P, 4], F32, name="bias_sb")  # bq m0,m1, bk m0,m1
        bv_sb = consts.tile([1, HD], MM_DT, name="bv_sb")
        wv_sb2 = wpool.tile([P, NKC, HD], MM_DT, name="wv_sb2")
        w8_sb = wpool.tile([P, 4, 2, 2 * HD], FP8, name="w8_sb")
        x8_sb = xpool.tile([P, 4, 2, T], FP8, name="x8_sb")
        wo_sb = wpool.tile([P, 2, C], MM_DT, name="wo_sb")
        xts = [
            xpool.tile([P, T], MM_DT, name=f"xt{kc}", tag=f"xt{kc}")
            for kc in range(NKC)
        ]
        # Q^T/K^T fp8 [P, 2, T]: k-tile 0 data, k-tile 1 zeros (DoubleRow pad)
        qt_sb = [qkv.tile([P, 2, T], FP8, name=f"qt{m}", tag=f"qt{m}") for m in range(2)]
        kt_sb = [qkv.tile([P, 2, T], FP8, name=f"kt{m}", tag=f"kt{m}") for m in range(2)]
        # V bf16 [P, ic, head, D+1]: ones in col D
        v_sb = qkv.tile([P, NIC, HPC, D + 1], MM_DT, name="v_sb")
        ot_sb = [qkv.tile([P, T], MM_DT, name=f"ot{m}", tag=f"ot{m}") for m in range(2)]
        # exp pair buffers [P, 1024] bf16 (chunk jj at cols 512*jj for full
        # pairs; diagonal pairs packed contiguously), manually rotated
        exbufs = [
            qkv.tile([P, 1024], MM_DT, name=f"ex{i}", tag=f"ex{i}")
            for i in range(N_EX)
        ]
        ex_idx = [0]

        def next_ex():
            b = exbufs[ex_idx[0] % N_EX]
            ex_idx[0] += 1
            return b

        def wv_c(kc):
            return wv_sb2[:, kc, :]

        # Preload the ACT Exp table while the first DMAs stream.
        scx = consts.tile([1, 1], F32, name="scx")
        nc.vector.memset(scx, 0.0)
        scy = consts.tile([1, 1], F32, name="scy")
        nc.scalar.activation(scy, scx, AF.Exp)

        # ---- loads ----
        # All input DMAs ride the SP queue (its sequencer is otherwise idle;
        # DMAs on the ACT queue would serialize ahead of the first exp).
        qs.dma_start(w8_sb, WQK8.rearrange("(a b p) c -> p a b c", p=P, b=2))
        qs.dma_start(bias_sb, BQK)
        for kc in range(NKC):
            qs.dma_start(
                x8_sb[:, kc // 2, kc % 2, 0:512],
                XT8[P * kc : P * (kc + 1), 0:512],
            )
        qs.dma_start(m01_sb, M01)
        qs.dma_start(ones_sb, ONES)
        for kc in range(NKC):
            qs.dma_start(wv_sb2[:, kc, :], WV[P * kc : P * (kc + 1), :])
        for kc in range(NKC):
            qs.dma_start(xts[kc][:, 0:512], XT[kc * P : (kc + 1) * P, 0:512])
        qs.dma_start(idn_sb, IDN)
        qs.dma_start(bv_sb, BV)
        qs.dma_start(v_sb[:, :, :, D : D + 1], VONES)
        for kc in range(NKC):
            qs.dma_start(
                x8_sb[:, kc // 2, kc % 2, 512:T], XT8[P * kc : P * (kc + 1), 512:T]
            )
        for t in range(1, NT):
            sl_ = slice(512 * t, 512 * (t + 1))
            for kc in range(NKC):
                qs.dma_start(xts[kc][:, sl_], XT[kc * P : (kc + 1) * P, sl_])
        qs.dma_start(wo_sb, WO.rearrange("(a p) c -> p a c", p=P))

        psum = ctx.enter_context(tc.tile_pool(name="psum", bufs=2, space="PSUM"))
        npool = ctx.enter_context(tc.tile_pool(name="npool", bufs=2))
        opool = ctx.enter_context(tc.tile_pool(name="opool", bufs=3))

        # ---- stage A: projections for one i-tile ----
        def stage_a(t):
            sl = slice(512 * t, 512 * (t + 1))
            for m in range(2):
                msl = slice(P * m, P * (m + 1))
                pqk = psum.tile([P, 1024], F32, tag="big", bufs=3)
                for kc in range(NKC):
                    nc.tensor.matmul(
                        pqk[:, 0:512],
                        lhsT=wq_c(kc, msl),
                        rhs=xts[kc][:, sl],
                        start=(kc == 0),
                        stop=(kc == NKC - 1),
                    )
                    nc.tensor.matmul(
                        pqk[:, 512:1024],
                        lhsT=wk_c(kc, msl),
                        rhs=xts[kc][:, sl],
                        start=(kc == 0),
                        stop=(kc == NKC - 1),
                    )
                with nc.allow_low_precision(reason="fp8 scores"):
                    nc.vector.tensor_scalar_add(
                        qt_sb[m][:, 0, sl], pqk[:, 0:512], bias_sb[:, m : m + 1]
                    )
                    nc.vector.tensor_scalar_add(
                        kt_sb[m][:, 0, sl], pqk[:, 512:1024], bias_sb[:, 2 + m : 3 + m]
                    )
            for ic in range(4 * t, 4 * (t + 1)):
                isl = slice(P * ic, P * (ic + 1))
                pv = psum.tile([P, HD], F32, tag="po", bufs=2)
                for kc in range(NKC):
                    nc.tensor.matmul(
                        pv,
                        lhsT=xts[kc][:, isl],
                        rhs=wv_c(kc),
                        start=(kc == 0),
                        stop=False,
                    )
                nc.tensor.matmul(
                    pv, lhsT=ones_sb, rhs=bv_sb, start=False, stop=True
                )
                with nc.allow_low_precision(reason="fp8 AV"):
                    nc.vector.tensor_copy(
                        v_sb[:, ic, :, 0:D], pv.rearrange("p (h d) -> p h d", d=D)
                    )

        # One-time zeroing of the DoubleRow pad k-tiles (emitted after
        # stage_a(0) so the first bias-adds aren't delayed on DVE).
        def pad_zeros():
            nc.vector.memset(qt_sb[0][:, 1, :], 0.0)
            nc.vector.memset(kt_sb[0][:, 1, :], 0.0)
            nc.gpsimd.memset(qt_sb[1][:, 1, :], 0.0)
            nc.gpsimd.memset(kt_sb[1][:, 1, :], 0.0)

        # ---- stage B: attention for one i-tile ----
        # Emission is software-pipelined: each pair-unit emits its scores+exp
        # immediately, its AV matmuls one unit later (so they never jam the
        # 4-deep PE wait queue behind a pending exp), and the per-head-pair
        # normalize/transpose work one more unit later. `fillers` (stage A/C
        # units of other tiles) are interspersed between pair-units so the PE
        # stream has dense independent work during the ACT-bound stretches.
        widths = {0: 512, 1: 384, 2: 256, 3: 128}
        on2_ref = [None, None]
        deferred = [None]   # carried AV+normalize block, may cross tiles
        head_unit = [0]

        def stage_b(t, fillers, per_tick=1):
            sl = slice(512 * t, 512 * (t + 1))
            n_units = (2 * t + 2) * HPC
            fill_every = (
                max(1, (n_units * per_tick) // max(1, len(fillers))) if fillers else 0
            )
            unit_no = [0]
            pending = []  # deferred transpose blocks, emitted one unit later

            head_unit = [0]

            def tick():
                unit_no[0] += 1
                head_unit[0] += 1
                if head_unit[0] == 2 and deferred[0] is not None:
                    deferred[0]()
                    deferred[0] = None
                if pending:
                    pending.pop(0)()
                if unit_no[0] % fill_every == 0 if fillers else False:
                    for _ in range(min(per_tick, len(fillers))):
                        fillers.pop(0)()

            for l in range(HPC):
                mc, ro = l // 2, 64 * (l % 2)
                qrow = slice(ro, ro + 64)
                if l % 2 == 0:
                    on2 = npool.tile(
                        [P, 4, 2, D], MM_DT, name=f"on{mc}", tag=f"on{mc}", bufs=2
                    )
                    on2_ref[mc] = on2
                else:
                    on2 = on2_ref[mc]

                # scores + exp stream; each pair's exp'd chunks stay live in
                # their exbuf until this head's AV accumulations consume them
                exs = []  # exbuf per pair, chunk jj at column 512*jj (full)
                for jcp in range(2 * t):
                    ps = psum.tile([P, 1024], F32, name="ps", tag="big", bufs=3)
                    for half in (0, 1):
                        jc = 2 * jcp + half
                        nc.tensor.matmul(
                            ps[:, 512 * half : 512 * (half + 1)],
                            lhsT=kt_sb[mc][qrow, :, P * jc : P * (jc + 1)],
                            rhs=qt_sb[mc][qrow, :, sl],
                            start=True,
                            stop=True,
                            perf_mode=DR,
                        )
                    exb = next_ex()
                    with nc.allow_low_precision(reason="fp8 AV"):
                        nc.scalar.activation(exb, ps, AF.Exp)
                    exs.append(exb)
                    tick()

                # diagonal chunk pairs: chunk k covers tile-local queries
                # [128k, 512), packed contiguously in PSUM/exbuf
                for ka, kb in ((0, 1), (2, 3)):
                    wa, wb = widths[ka], widths[kb]
                    ps = psum.tile([P, 1024], F32, name="ps", tag="big", bufs=3)
                    for base, k, w in ((0, ka, wa), (wa, kb, wb)):
                        nc.tensor.matmul(
                            ps[:, base : base + w],
                            lhsT=kt_sb[mc][qrow, :, P * (4 * t + k) : P * (4 * t + k + 1)],
                            rhs=qt_sb[mc][qrow, :, 512 * (t + 1) - w : 512 * (t + 1)],
                            start=True,
                            stop=True,
                            perf_mode=DR,
                        )
                    exb = next_ex()
                    with nc.allow_low_precision(reason="fp8 AV"):
                        nc.scalar.activation(
                            exb[:, 0 : wa + wb], ps[:, 0 : wa + wb], AF.Exp
                        )
                    # causal mask on the leading 128x128 triangle blocks
                    # (post-exp multiplicative, on Pool: off the ACT path)
                    nc.gpsimd.tensor_mul(exb[:, 0:128], exb[:, 0:128], m01_sb)
                    nc.gpsimd.tensor_mul(
                        exb[:, wa : wa + 128], exb[:, wa : wa + 128], m01_sb
                    )
                    exs.append(exb)
                    tick()

                # AV per query chunk: serial accumulation into a bank-aligned
                # [P, 65] PSUM tile over chunks jc = 0 .. 4t+qi, then
                # normalize by the softmax sum (column D, per-partition).
                # Deferred one head so ACT streams the next head's exps while
                # the PE drains these accumulations.
                def av_block(l=l, mc=mc, on2=on2, exs=exs):
                    def ex_col(jc):
                        # (exbuf, column base of chunk jc's block)
                        if jc < 4 * t:
                            return exs[jc // 2], 512 * (jc % 2)
                        k = jc - 4 * t
                        if k < 2:
                            return exs[2 * t], 0 if k == 0 else widths[0]
                        return exs[2 * t + 1], 0 if k == 2 else widths[2]

                    for qi in range(4):
                        po_raw = psum.tile([P, 256], F32, name="po_raw", tag="po", bufs=1)
                        po = po_raw[:, 0 : D + 1]
                        for jc in range(4 * t + qi + 1):
                            exb, base = ex_col(jc)
                            k0 = max(0, jc - 4 * t)
                            off = base + 128 * (qi - k0)
                            nc.tensor.matmul(
                                po,
                                lhsT=exb[:, off : off + 128],
                                rhs=v_sb[:, jc, l, :],
                                start=(jc == 0),
                                stop=(jc == 4 * t + qi),
                                skip_group_check=True,
                            )
                        rc = npool.tile([P, 1], F32, name="rc", tag="rc", bufs=2)
                        nc.vector.reciprocal(rc, po[:, D : D + 1])
                        with nc.allow_low_precision(reason="bf16 out"):
                            nc.vector.tensor_scalar_mul(
                                on2[:, qi, l % 2, :], po[:, 0:D], rc
                            )
                    if l % 2 == 1:
                        for qi in range(4):
                            tp = psum.tile([P, P], MM_DT, name="tp", tag="tp", bufs=1)
                            nc.tensor.transpose(tp, on2[:, qi, :, :], idn_sb)
                            csl = slice(P * (4 * t + qi), P * (4 * t + qi + 1))
                            nc.vector.tensor_copy(ot_sb[mc][:, csl], tp)

                if DEFER_AV:
                    if deferred[0] is not None:
                        deferred[0]()
                    deferred[0] = av_block
                    head_unit[0] = 0
                else:
                    av_block()
            while pending:
                pending.pop(0)()
            while fillers:
                fillers.pop(0)()

        # ---- stage C: output projection, one unit per 128-row i-chunk ----
        def stage_c_unit(ic):
            def emit():
                isl = slice(P * ic, P * (ic + 1))
                ob = opool.tile([P, C], MM_DT, name="ob", tag="ob")
                pc = psum.tile([P, 1024], F32, name="pc", tag="big", bufs=3)
                for n in (0, 1):
                    for kc in range(2):
                        nc.tensor.matmul(
                            pc[:, 512 * n : 512 * (n + 1)],
                            lhsT=ot_sb[kc][:, isl],
                            rhs=wo_sb[:, kc, 512 * n : 512 * (n + 1)],
                            start=(kc == 0),
                            stop=(kc == 1),
                        )
                with nc.allow_low_precision(reason="bf16 out"):
                    nc.vector.tensor_copy(ob, pc)
                qs.dma_start(OUT[isl, :], ob)

            return emit

        def stage_a_units(t, which="qkv"):
            units = []
            sl = slice(512 * t, 512 * (t + 1))
            for m in range(2):
                def emit_qk(m=m):
                    msl = slice(P * m, P * (m + 1))
                    kmsl = slice(HD + P * m, HD + P * (m + 1))
                    pqk = psum.tile([P, 1024], F32, name="pqk", tag="big", bufs=3)
                    for kcp in range(4):
                        nc.tensor.matmul(
                            pqk[:, 0:512], lhsT=w8_sb[:, kcp, :, msl],
                            rhs=x8_sb[:, kcp, :, sl],
                            start=(kcp == 0), stop=(kcp == 3), perf_mode=DR,
                        )
                    for kcp in range(4):
                        nc.tensor.matmul(
                            pqk[:, 512:1024], lhsT=w8_sb[:, kcp, :, kmsl],
                            rhs=x8_sb[:, kcp, :, sl],
                            start=(kcp == 0), stop=(kcp == 3), perf_mode=DR,
                        )
                    mul, add = mybir.AluOpType.mult, mybir.AluOpType.add
                    with nc.allow_low_precision(reason="fp8 scores"):
                        nc.vector.tensor_scalar(
                            qt_sb[m][:, 0, sl], pqk[:, 0:512], 2.0 ** -6,
                            bias_sb[:, m : m + 1], mul, add,
                        )
                        nc.vector.tensor_scalar(
                            kt_sb[m][:, 0, sl], pqk[:, 512:1024], 2.0 ** -6,
                            bias_sb[:, 2 + m : 3 + m], mul, add,
                        )
                if "qk" in which:
                    units.append(emit_qk)
            for ic in range(4 * t, 4 * (t + 1)) if "v" in which else []:
                def emit_v(ic=ic):
                    isl = slice(P * ic, P * (ic + 1))
                    pv = psum.tile([P, HD], F32, name="pv", tag="po", bufs=1)
                    for kc in range(NKC):
                        nc.tensor.matmul(
                            pv, lhsT=xts[kc][:, isl], rhs=wv_c(kc),
                            start=(kc == 0), stop=False,
                        )
                    nc.tensor.matmul(pv, lhsT=ones_sb, rhs=bv_sb, start=False, stop=True)
                    with nc.allow_low_precision(reason="fp8 AV"):
                        nc.vector.tensor_copy(
                            v_sb[:, ic, :, 0:D],
                            pv.rearrange("p (h d) -> p h d", d=D),
                        )
                units.append(emit_v)
            return units

        # Emission: pads while the first DMAs stream, a(0), then attention
        # tiles in order 0,2,3,1 with later-stage units interleaved as PE
        # fillers inside the ACT-bound attention streams.
        pad_zeros()
        for u in stage_a_units(0, "qk"):
            u()
        stage_b(
            0,
            stage_a_units(0, "v") + stage_a_units(1) + stage_a_units(2),
            per_tick=2,
        )
        stage_b(2, stage_a_units(3) + [stage_c_unit(ic) for ic in range(0, 4)])
        stage_b(3, [stage_c_unit(ic) for ic in range(8, 12)])
        stage_b(1, [stage_c_unit(ic) for ic in range(12, 16)])
        if deferred[0] is not None:
            deferred[0]()
            deferred[0] = None
        for ic in range(4, 8):
            stage_c_unit(ic)()



def _get_program():
    if "nc" not in _CACHE:
        _CACHE["nc"] = _build_program()
    return _CACHE["nc"]


class _Runner:
    """Reusable SPMD executor (adapted from concourse.bass2jax.run_bass_via_pjrt)
    so repeated kernel() calls reuse one compiled executable."""

    def __init__(self, nc):
        import jax
        import concourse.mybir as mb
        from jax.sharding import Mesh, PartitionSpec
        from jax.experimental.shard_map import shard_map
        from concourse import bass2jax

        bass2jax.install_neuronx_cc_hook()
        self.jax = jax
        self.nc = nc
        partition_name = (
            nc.partition_id_tensor.name if nc.partition_id_tensor else None
        )
        in_names, out_names, out_avals, zero_outs = [], [], [], []
        for alloc in nc.m.functions[0].allocations:
            if not isinstance(alloc, mb.MemoryLocationSet):
                continue
            name = alloc.memorylocations[0].name
            if alloc.kind == "ExternalInput":
                if name != partition_name:
                    in_names.append(name)
            elif alloc.kind == "ExternalOutput":
                shape = tuple(alloc.tensor_shape)
                dtype = mb.dt.np(alloc.dtype)
                out_names.append(name)
                out_avals.append(jax.core.ShapedArray(shape, dtype))
                zero_outs.append((shape, dtype))
        self.n_params = len(in_names)
        self.in_names = list(in_names)
        self.out_names = out_names
        self.out_avals = out_avals
        self.zero_outs = zero_outs
        all_in_names = in_names + out_names + (
            [partition_name] if partition_name else []
        )
        donate = tuple(range(self.n_params, self.n_params + len(out_names)))

        def _body(*args):
            operands = list(args)
            if partition_name is not None:
                operands.append(bass2jax.partition_id_tensor())
            outs = bass2jax._bass_exec_p.bind(
                *operands,
                out_avals=tuple(out_avals),
                in_names=tuple(all_in_names),
                out_names=tuple(out_names),
                lowering_input_output_aliases=(),
                sim_require_finite=True,
                sim_require_nnan=True,
                nc=nc,
            )
            return tuple(outs)

        devices = jax.devices()[:N_CORES]
        self.mesh = Mesh(np.asarray(devices), ("core",))
        in_specs = (PartitionSpec("core"),) * (self.n_params + len(out_names))
        out_specs = (PartitionSpec("core"),) * len(out_names)
        self.sharded = jax.jit(
            shard_map(
                _body,
                mesh=self.mesh,
                in_specs=in_specs,
                out_specs=out_specs,
                check_rep=False,
            ),
            donate_argnums=donate,
            keep_unused=True,
        )

    def concat_inputs(self, in_maps):
        return [
            np.concatenate([np.asarray(m[name]) for m in in_maps], axis=0)
            for name in self.in_names
        ]

    def zeros(self):
        return [
            np.zeros((N_CORES * s[0], *s[1:]), d) for s, d in self.zero_outs
        ]

    def run(self, concat_in, zeros):
        out_arrs = self.sharded(*concat_in, *zeros)
        return out_arrs

    def split(self, out_arrs):
        res = []
        for c in range(N_CORES):
            res.append(
                {
                    name: np.asarray(out_arrs[i]).reshape(
                        N_CORES, *self.out_avals[i].shape
                    )[c]
                    for i, name in enumerate(self.out_names)
                }
            )
        return res


def _get_runner():
    if "runner" not in _CACHE:
        _CACHE["runner"] = _Runner(_get_program())
    return _CACHE["runner"]


def _shard_inputs(X, Wq, bq, Wk, bk, Wv, bv, Wo, bo):
    import ml_dtypes

    bf16 = ml_dtypes.bfloat16
    in_maps = []
    for c in range(N_CORES):
        b, hg = divmod(c, HG)
        cols = slice(HD * hg, HD * (hg + 1))
        sq = 2.0 ** -1.5  # split 1/sqrt(D)=1/8 over Q and K for fp8 range
        bqk = np.stack(
            [
                bq[cols][:P] * sq,
                bq[cols][P:] * sq,
                bk[cols][:P] * sq,
                bk[cols][P:] * sq,
            ],
            axis=1,
        ).astype(np.float32)
        f8 = ml_dtypes.float8_e4m3
        xt = np.ascontiguousarray(X[b].T)
        in_maps.append(
            {
                "XT": xt.astype(bf16),
                "XT8": xt.astype(f8),
                "WQK8": np.concatenate(
                    [Wq[:, cols] * (sq * 64), Wk[:, cols] * (sq * 64)], axis=1
                ).astype(f8),
                "WV": Wv[:, cols].astype(bf16),
                "BQK": bqk,
                "BV": bv[cols].reshape(1, HD).astype(bf16),
                "WO": np.ascontiguousarray(Wo[cols, :]).astype(bf16),
            }
        )
    return in_maps


def kernel(X, Wq, bq, Wk, bk, Wv, bv, Wo, bo):
    X = np.asarray(X, dtype=np.float32)
    Wq, bq = np.asarray(Wq, np.float32), np.asarray(bq, np.float32)
    Wk, bk = np.asarray(Wk, np.float32), np.asarray(bk, np.float32)
    Wv, bv = np.asarray(Wv, np.float32), np.asarray(bv, np.float32)
    Wo, bo = np.asarray(Wo, np.float32), np.asarray(bo, np.float32)

    runner = _get_runner()
    in_maps = _shard_inputs(X, Wq, bq, Wk, bk, Wv, bv, Wo, bo)
    res = runner.split(runner.run(runner.concat_inputs(in_maps), runner.zeros()))

    out = np.empty((B, T, C), dtype=np.float32)
    for b in range(B):
        acc = np.zeros((T, C), dtype=np.float64)
        for hg in range(HG):
            acc += res[HG * b + hg]["OUT"].astype(np.float64)
        out[b] = (acc + bo.astype(np.float64)).astype(np.float32)
    return out


# revision 59
# speedup vs baseline: 1.0287x; 1.0287x over previous
"""Causal multi-head attention block (B=2, T=2048, C=1024, H=16) on 8 TRN2 cores.

Sharding: tensor-parallel over heads x data-parallel over batch.
Core c handles batch b = c // 4 and head-group hg = c % 4 (4 heads = 256 of
the 1024 channel columns). Each core computes, for its batch and heads:
    QT/KT = (Wqk/8^0.5-ish)^T X^T + b  (fp8e4m3 DoubleRow matmuls; host sends
            fp8 copies of X^T and 64x-scaled Wq|Wk, rescaled 2^-6 in the
            fused bias step; 1/sqrt(D) split over Q and K for fp8 range)
    V     = X Wv + bv            (bf16 matmuls, bf16 V + ones column)
    S^T   = K Q^T per 128-key chunk (fp8 DoubleRow, zero-padded 2nd k-tile)
    P^T   = exp(S^T) -> bf16; causal mask applied post-exp as a
            multiplicative 0/1 [128,128] block on the Pool engine
    O     = P V per 128-query chunk (bf16): out[q,0:64]=sum(P*V),
            out[q,64]=sum(P); per-partition softmax normalize
    O^T   via PE transpose (identity matmul) of the [q, 2*64] head-pair block
    partial = O^T rows @ Wo_rows_slice -> OUT bf16 [2048, 1024]
Host sums the 4 partials per batch and adds bo.

Schedule notes (engines execute their streams IN ORDER; emission = schedule):
 - all input DMAs ride the SP queue (ACT-queue DMAs would serialize ahead of
   the exps in the ACT sequencer stream)
 - each head's AV+normalize block is deferred past the next head's first two
   score pairs so ACT streams exps continuously
 - stage-A/C units are interleaved as PE fillers inside the attention streams
 - matmul PSUM outputs must be bank-aligned; transposes need their own
   PSUM tag (sharing the po slot corrupts under rotation)
"""

from contextlib import ExitStack

import numpy as np

import concourse.bacc as bacc
import concourse.mybir as mybir
import concourse.tile as tile
from concourse.bass_utils import run_bass_kernel_spmd

B, T, C, H, D = 2, 2048, 1024, 16, 64
N_CORES = 8
HG = 4                  # head-groups (tensor parallel)
HPC = H // HG           # heads per core = 4
HD = HPC * D            # channel slice per core = 256
P = 128                 # partitions
NT = T // 512           # 4 i-tiles of 512
NIC = T // P            # 16 i-chunks of 128
NKC = C // P            # 8 contraction chunks of 128
F32 = mybir.dt.float32
FP8 = mybir.dt.float8e4
DR = mybir.MatmulPerfMode.DoubleRow
AF = mybir.ActivationFunctionType

MM_DT = mybir.dt.bfloat16   # projection/output matmul dtype
N_EX = 26                   # exp-pair buffers (cross-tile AV deferral liveness)
DEFER_AV = True             # emit each head's AV block after next head's scores

_CACHE: dict = {}


def _build_program():
    import ml_dtypes

    bf16 = ml_dtypes.bfloat16
    nc = bacc.Bacc("TRN2", debug=False)

    XT = nc.dram_tensor("XT", [C, T], MM_DT, kind="ExternalInput").ap()
    XT8 = nc.dram_tensor("XT8", [C, T], FP8, kind="ExternalInput").ap()
    WQK8 = nc.dram_tensor("WQK8", [C, 2 * HD], FP8, kind="ExternalInput").ap()
    WV = nc.dram_tensor("WV", [C, HD], MM_DT, kind="ExternalInput").ap()
    BQK = nc.dram_tensor("BQK", [P, 4], F32, kind="ExternalInput").ap()
    BV = nc.dram_tensor("BV", [1, HD], MM_DT, kind="ExternalInput").ap()
    WO = nc.dram_tensor("WO", [HD, C], MM_DT, kind="ExternalInput").ap()
    OUT = nc.dram_tensor("OUT", [T, C], MM_DT, kind="ExternalOutput").ap()


    # Multiplicative causal mask for the diagonal 128x128 block of each
    # diagonal key-chunk: element (p, j) valid iff j >= p.
    m01 = (np.arange(128)[None, :] >= np.arange(128)[:, None]).astype(bf16)
    M01 = nc.inline_tensor(m01, name="m01").ap()
    IDN = nc.inline_tensor(np.eye(128, dtype=bf16), name="idn").ap()
    ONES = nc.inline_tensor(np.ones((1, P), bf16), name="ones_c").ap()
    VONES = nc.inline_tensor(np.ones((P, NIC * HPC), bf16), name="vones_c").ap()

    with tile.TileContext(nc) as tc:
        _trace_kernel(tc, XT, XT8, WQK8, WV, BQK, BV, WO, OUT, M01, IDN, ONES, VONES)
    nc.compile()
    return nc


def _trace_kernel(tc, XT, XT8, WQK8, WV, BQK, BV, WO, OUT, M01, IDN, ONES, VONES):
    nc = tc.nc

    with ExitStack() as ctx:
        consts = ctx.enter_context(tc.tile_pool(name="consts", bufs=1))
        wpool = ctx.enter_context(tc.tile_pool(name="weights", bufs=1))
        xpool = ctx.enter_context(tc.tile_pool(name="xt", bufs=1))
        qkv = ctx.enter_context(tc.tile_pool(name="qkv", bufs=1))

        # Two HWDGE queues: SP (nc.sync) and Activation (nc.scalar).
        qs, qa = nc.sync, nc.scalar

        # ---- tiles ----
        m01_sb = consts.tile([P, P], MM_DT, name="m01_sb")
        idn_sb = consts.tile([P, P], MM_DT, name="idn_sb")
        ones_sb = consts.tile([1, P], MM_DT, name="ones_sb")
        bias_sb = consts.tile([P, 4], F32, name="bias_sb")  # bq m0,m1, bk m0,m1
        bv_sb = consts.tile([1, HD], MM_DT, name="bv_sb")
        wv_sb2 = wpool.tile([P, NKC, HD], MM_DT, name="wv_sb2")
        w8_sb = wpool.tile([P, 4, 2, 2 * HD], FP8, name="w8_sb")
        x8_sb = xpool.tile([P, 4, 2, T], FP8, name="x8_sb")
        wo_sb = wpool.tile([P, 2, C], MM_DT, name="wo_sb")
        xts = [
            xpool.tile([P, T], MM_DT, name=f"xt{kc}", tag=f"xt{kc}")
            for kc in range(NKC)
        ]
        # Q^T/K^T fp8 [P, 2, T]: k-tile 0 data, k-tile 1 zeros (DoubleRow pad)
        qt_sb = [qkv.tile([P, 2, T], FP8, name=f"qt{m}", tag=f"qt{m}") for m in range(2)]
        kt_sb = [qkv.tile([P, 2, T], FP8, name=f"kt{m}", tag=f"kt{m}") for m in range(2)]
        # V bf16 [P, ic, head, D+1]: ones in col D
        v_sb = qkv.tile([P, NIC, HPC, D + 1], MM_DT, name="v_sb")
        ot_sb = [qkv.tile([P, T], MM_DT, name=f"ot{m}", tag=f"ot{m}") for m in range(2)]
        # exp pair buffers [P, 1024] bf16 (chunk jj at cols 512*jj for full
        # pairs; diagonal pairs packed contiguously), manually rotated
        exbufs = [
            qkv.tile([P, 1024], MM_DT, name=f"ex{i}", tag=f"ex{i}")
            for i in range(N_EX)
        ]
        ex_idx = [0]

        def next_ex():
            b = exbufs[ex_idx[0] % N_EX]
            ex_idx[0] += 1
            return b

        def wv_c(kc):
            return wv_sb2[:, kc, :]

        # Preload the ACT Exp table while the first DMAs stream.
        scx = consts.tile([1, 1], F32, name="scx")
        nc.vector.memset(scx, 0.0)
        scy = consts.tile([1, 1], F32, name="scy")
        nc.scalar.activation(scy, scx, AF.Exp)

        # ---- loads ----
        # All input DMAs ride the SP queue (its sequencer is otherwise idle;
        # DMAs on the ACT queue would serialize ahead of the first exp).
        qs.dma_start(w8_sb, WQK8.rearrange("(a b p) c -> p a b c", p=P, b=2))
        qs.dma_start(bias_sb, BQK)
        for kc in range(NKC):
            qs.dma_start(
                x8_sb[:, kc // 2, kc % 2, 0:512],
                XT8[P * kc : P * (kc + 1), 0:512],
            )
        qs.dma_start(m01_sb, M01)
        qs.dma_start(ones_sb, ONES)
        qs.dma_start(wv_sb2, WV.rearrange("(a p) c -> p a c", p=P))
        for kc in range(NKC):
            qs.dma_start(xts[kc][:, 0:512], XT[kc * P : (kc + 1) * P, 0:512])
        qs.dma_start(idn_sb, IDN)
        qs.dma_start(bv_sb, BV)
        qs.dma_start(v_sb[:, :, :, D : D + 1], VONES)
        for kc in range(NKC):
            qs.dma_start(
                x8_sb[:, kc // 2, kc % 2, 512:T], XT8[P * kc : P * (kc + 1), 512:T]
            )
        for t in range(1, NT):
            sl_ = slice(512 * t, 512 * (t + 1))
            for kc in range(NKC):
                qs.dma_start(xts[kc][:, sl_], XT[kc * P : (kc + 1) * P, sl_])
        qs.dma_start(wo_sb, WO.rearrange("(a p) c -> p a c", p=P))

        psum = ctx.enter_context(tc.tile_pool(name="psum", bufs=2, space="PSUM"))
        npool = ctx.enter_context(tc.tile_pool(name="npool", bufs=2))
        opool = ctx.enter_context(tc.tile_pool(name="opool", bufs=3))

        # ---- stage A: projections for one i-tile ----
        def stage_a(t):
            sl = slice(512 * t, 512 * (t + 1))
            for m in range(2):
                msl = slice(P * m, P * (m + 1))
                pqk = psum.tile([P, 1024], F32, tag="big", bufs=3)
                for kc in range(NKC):
                    nc.tensor.matmul(
                        pqk[:, 0:512],
                        lhsT=wq_c(kc, msl),
                        rhs=xts[kc][:, sl],
                        start=(kc == 0),
                        stop=(kc == NKC - 1),
                    )
                    nc.tensor.matmul(
                        pqk[:, 512:1024],
                        lhsT=wk_c(kc, msl),
                        rhs=xts[kc][:, sl],
                        start=(kc == 0),
                        stop=(kc == NKC - 1),
                    )
                with nc.allow_low_precision(reason="fp8 scores"):
                    nc.vector.tensor_scalar_add(
                        qt_sb[m][:, 0, sl], pqk[:, 0:512], bias_sb[:, m : m + 1]
                    )
                    nc.vector.tensor_scalar_add(
                        kt_sb[m][:, 0, sl], pqk[:, 512:1024], bias_sb[:, 2 + m : 3 + m]
                    )
            for ic in range(4 * t, 4 * (t + 1)):
                isl = slice(P * ic, P * (ic + 1))
                pv = psum.tile([P, HD], F32, tag="po", bufs=2)
                for kc in range(NKC):
                    nc.tensor.matmul(
                        pv,
                        lhsT=xts[kc][:, isl],
                        rhs=wv_c(kc),
                        start=(kc == 0),
                        stop=False,
                    )
                nc.tensor.matmul(
                    pv, lhsT=ones_sb, rhs=bv_sb, start=False, stop=True
                )
                with nc.allow_low_precision(reason="fp8 AV"):
                    nc.vector.tensor_copy(
                        v_sb[:, ic, :, 0:D], pv.rearrange("p (h d) -> p h d", d=D)
                    )

        # One-time zeroing of the DoubleRow pad k-tiles (emitted after
        # stage_a(0) so the first bias-adds aren't delayed on DVE).
        def pad_zeros():
            nc.vector.memset(qt_sb[0][:, 1, :], 0.0)
            nc.vector.memset(kt_sb[0][:, 1, :], 0.0)
            nc.gpsimd.memset(qt_sb[1][:, 1, :], 0.0)
            nc.gpsimd.memset(kt_sb[1][:, 1, :], 0.0)

        # ---- stage B: attention for one i-tile ----
        # Emission is software-pipelined: each pair-unit emits its scores+exp
        # immediately, its AV matmuls one unit later (so they never jam the
        # 4-deep PE wait queue behind a pending exp), and the per-head-pair
        # normalize/transpose work one more unit later. `fillers` (stage A/C
        # units of other tiles) are interspersed between pair-units so the PE
        # stream has dense independent work during the ACT-bound stretches.
        widths = {0: 512, 1: 384, 2: 256, 3: 128}
        on2_ref = [None, None]
        deferred = [None]   # carried AV+normalize block, may cross tiles
        head_unit = [0]

        def stage_b(t, fillers, per_tick=1):
            sl = slice(512 * t, 512 * (t + 1))
            n_units = (2 * t + 2) * HPC
            fill_every = (
                max(1, (n_units * per_tick) // max(1, len(fillers))) if fillers else 0
            )
            unit_no = [0]
            pending = []  # deferred transpose blocks, emitted one unit later

            head_unit = [0]

            def tick():
                unit_no[0] += 1
                head_unit[0] += 1
                if head_unit[0] == 2 and deferred[0] is not None:
                    deferred[0]()
                    deferred[0] = None
                if pending:
                    pending.pop(0)()
                if unit_no[0] % fill_every == 0 if fillers else False:
                    for _ in range(min(per_tick, len(fillers))):
                        fillers.pop(0)()

            for l in range(HPC):
                mc, ro = l // 2, 64 * (l % 2)
                qrow = slice(ro, ro + 64)
                if l % 2 == 0:
                    on2 = npool.tile(
                        [P, 4, 2, D], MM_DT, name=f"on{mc}", tag=f"on{mc}", bufs=2
                    )
                    on2_ref[mc] = on2
                else:
                    on2 = on2_ref[mc]

                # scores + exp stream; each pair's exp'd chunks stay live in
                # their exbuf until this head's AV accumulations consume them
                exs = []  # exbuf per pair, chunk jj at column 512*jj (full)
                for jcp in range(2 * t):
                    ps = psum.tile([P, 1024], F32, name="ps", tag="big", bufs=3)
                    for half in (0, 1):
                        jc = 2 * jcp + half
                        nc.tensor.matmul(
                            ps[:, 512 * half : 512 * (half + 1)],
                            lhsT=kt_sb[mc][qrow, :, P * jc : P * (jc + 1)],
                            rhs=qt_sb[mc][qrow, :, sl],
                            start=True,
                            stop=True,
                            perf_mode=DR,
                        )
                    exb = next_ex()
                    with nc.allow_low_precision(reason="fp8 AV"):
                        nc.scalar.activation(exb, ps, AF.Exp)
                    exs.append(exb)
                    tick()

                # diagonal chunk pairs: chunk k covers tile-local queries
                # [128k, 512), packed contiguously in PSUM/exbuf
                for ka, kb in ((0, 1), (2, 3)):
                    wa, wb = widths[ka], widths[kb]
                    ps = psum.tile([P, 1024], F32, name="ps", tag="big", bufs=3)
                    for base, k, w in ((0, ka, wa), (wa, kb, wb)):
                        nc.tensor.matmul(
                            ps[:, base : base + w],
                            lhsT=kt_sb[mc][qrow, :, P * (4 * t + k) : P * (4 * t + k + 1)],
                            rhs=qt_sb[mc][qrow, :, 512 * (t + 1) - w : 512 * (t + 1)],
                            start=True,
                            stop=True,
                            perf_mode=DR,
                        )
                    exb = next_ex()
                    with nc.allow_low_precision(reason="fp8 AV"):
                        nc.scalar.activation(
                            exb[:, 0 : wa + wb], ps[:, 0 : wa + wb], AF.Exp
                        )
                    # causal mask on the leading 128x128 triangle blocks
                    # (post-exp multiplicative, on Pool: off the ACT path)
                    nc.gpsimd.tensor_mul(exb[:, 0:128], exb[:, 0:128], m01_sb)
                    nc.gpsimd.tensor_mul(
                        exb[:, wa : wa + 128], exb[:, wa : wa + 128], m01_sb
                    )
                    exs.append(exb)
                    tick()

                # AV per query chunk: serial accumulation into a bank-aligned
                # [P, 65] PSUM tile over chunks jc = 0 .. 4t+qi, then
                # normalize by the softmax sum (column D, per-partition).
                # Deferred one head so ACT streams the next head's exps while
                # the PE drains these accumulations.
                def av_block(l=l, mc=mc, on2=on2, exs=exs):
                    def ex_col(jc):
                        # (exbuf, column base of chunk jc's block)
                        if jc < 4 * t:
                            return exs[jc // 2], 512 * (jc % 2)
                        k = jc - 4 * t
                        if k < 2:
                            return exs[2 * t], 0 if k == 0 else widths[0]
                        return exs[2 * t + 1], 0 if k == 2 else widths[2]

                    for qi in range(4):
                        po_raw = psum.tile([P, 256], F32, name="po_raw", tag="po", bufs=1)
                        po = po_raw[:, 0 : D + 1]
                        for jc in range(4 * t + qi + 1):
                            exb, base = ex_col(jc)
                            k0 = max(0, jc - 4 * t)
                            off = base + 128 * (qi - k0)
                            nc.tensor.matmul(
                                po,
                                lhsT=exb[:, off : off + 128],
                                rhs=v_sb[:, jc, l, :],
                                start=(jc == 0),
                                stop=(jc == 4 * t + qi),
                                skip_group_check=True,
                            )
                        rc = npool.tile([P, 1], F32, name="rc", tag="rc", bufs=2)
                        nc.vector.reciprocal(rc, po[:, D : D + 1])
                        with nc.allow_low_precision(reason="bf16 out"):
                            nc.vector.tensor_scalar_mul(
                                on2[:, qi, l % 2, :], po[:, 0:D], rc
                            )
                    if l % 2 == 1:
                        for qi in range(4):
                            tp = psum.tile([P, P], MM_DT, name="tp", tag="tp", bufs=1)
                            nc.tensor.transpose(tp, on2[:, qi, :, :], idn_sb)
                            csl = slice(P * (4 * t + qi), P * (4 * t + qi + 1))
                            nc.vector.tensor_copy(ot_sb[mc][:, csl], tp)

                if DEFER_AV:
                    if deferred[0] is not None:
                        deferred[0]()
                    deferred[0] = av_block
                    head_unit[0] = 0
                else:
                    av_block()
            while pending:
                pending.pop(0)()
            while fillers:
                fillers.pop(0)()

        # ---- stage C: output projection, one unit per 128-row i-chunk ----
        def stage_c_unit(ic):
            def emit():
                isl = slice(P * ic, P * (ic + 1))
                ob = opool.tile([P, C], MM_DT, name="ob", tag="ob")
                pc = psum.tile([P, 1024], F32, name="pc", tag="big", bufs=3)
                for n in (0, 1):
                    for kc in range(2):
                        nc.tensor.matmul(
                            pc[:, 512 * n : 512 * (n + 1)],
                            lhsT=ot_sb[kc][:, isl],
                            rhs=wo_sb[:, kc, 512 * n : 512 * (n + 1)],
                            start=(kc == 0),
                            stop=(kc == 1),
                        )
                with nc.allow_low_precision(reason="bf16 out"):
                    nc.vector.tensor_copy(ob, pc)
                qs.dma_start(OUT[isl, :], ob)

            return emit

        def stage_a_units(t, which="qkv"):
            units = []
            sl = slice(512 * t, 512 * (t + 1))
            for m in range(2):
                def emit_qk(m=m):
                    msl = slice(P * m, P * (m + 1))
                    kmsl = slice(HD + P * m, HD + P * (m + 1))
                    pqk = psum.tile([P, 1024], F32, name="pqk", tag="big", bufs=3)
                    for kcp in range(4):
                        nc.tensor.matmul(
                            pqk[:, 0:512], lhsT=w8_sb[:, kcp, :, msl],
                            rhs=x8_sb[:, kcp, :, sl],
                            start=(kcp == 0), stop=(kcp == 3), perf_mode=DR,
                        )
                    for kcp in range(4):
                        nc.tensor.matmul(
                            pqk[:, 512:1024], lhsT=w8_sb[:, kcp, :, kmsl],
                            rhs=x8_sb[:, kcp, :, sl],
                            start=(kcp == 0), stop=(kcp == 3), perf_mode=DR,
                        )
                    mul, add = mybir.AluOpType.mult, mybir.AluOpType.add
                    with nc.allow_low_precision(reason="fp8 scores"):
                        nc.vector.tensor_scalar(
                            qt_sb[m][:, 0, sl], pqk[:, 0:512], 2.0 ** -6,
                            bias_sb[:, m : m + 1], mul, add,
                        )
                        nc.vector.tensor_scalar(
                            kt_sb[m][:, 0, sl], pqk[:, 512:1024], 2.0 ** -6,
                            bias_sb[:, 2 + m : 3 + m], mul, add,
                        )
                if "qk" in which:
                    units.append(emit_qk)
            for ic in range(4 * t, 4 * (t + 1)) if "v" in which else []:
                def emit_v(ic=ic):
                    isl = slice(P * ic, P * (ic + 1))
                    pv = psum.tile([P, HD], F32, name="pv", tag="po", bufs=1)
                    for kc in range(NKC):
                        nc.tensor.matmul(
                            pv, lhsT=xts[kc][:, isl], rhs=wv_c(kc),
                            start=(kc == 0), stop=False,
                        )
                    nc.tensor.matmul(pv, lhsT=ones_sb, rhs=bv_sb, start=False, stop=True)
                    with nc.allow_low_precision(reason="fp8 AV"):
                        nc.vector.tensor_copy(
                            v_sb[:, ic, :, 0:D],
                            pv.rearrange("p (h d) -> p h d", d=D),
                        )
                units.append(emit_v)
            return units

        # Emission: pads while the first DMAs stream, a(0), then attention
        # tiles in order 0,2,3,1 with later-stage units interleaved as PE
        # fillers inside the ACT-bound attention streams.
        pad_zeros()
        for u in stage_a_units(0, "qk"):
            u()
        stage_b(
            0,
            stage_a_units(0, "v") + stage_a_units(1) + stage_a_units(2),
            per_tick=2,
        )
        stage_b(2, stage_a_units(3) + [stage_c_unit(ic) for ic in range(0, 4)])
        stage_b(3, [stage_c_unit(ic) for ic in range(8, 12)])
        stage_b(1, [stage_c_unit(ic) for ic in range(12, 16)])
        if deferred[0] is not None:
            deferred[0]()
            deferred[0] = None
        for ic in range(4, 8):
            stage_c_unit(ic)()



def _get_program():
    if "nc" not in _CACHE:
        _CACHE["nc"] = _build_program()
    return _CACHE["nc"]


class _Runner:
    """Reusable SPMD executor (adapted from concourse.bass2jax.run_bass_via_pjrt)
    so repeated kernel() calls reuse one compiled executable."""

    def __init__(self, nc):
        import jax
        import concourse.mybir as mb
        from jax.sharding import Mesh, PartitionSpec
        from jax.experimental.shard_map import shard_map
        from concourse import bass2jax

        bass2jax.install_neuronx_cc_hook()
        self.jax = jax
        self.nc = nc
        partition_name = (
            nc.partition_id_tensor.name if nc.partition_id_tensor else None
        )
        in_names, out_names, out_avals, zero_outs = [], [], [], []
        for alloc in nc.m.functions[0].allocations:
            if not isinstance(alloc, mb.MemoryLocationSet):
                continue
            name = alloc.memorylocations[0].name
            if alloc.kind == "ExternalInput":
                if name != partition_name:
                    in_names.append(name)
            elif alloc.kind == "ExternalOutput":
                shape = tuple(alloc.tensor_shape)
                dtype = mb.dt.np(alloc.dtype)
                out_names.append(name)
                out_avals.append(jax.core.ShapedArray(shape, dtype))
                zero_outs.append((shape, dtype))
        self.n_params = len(in_names)
        self.in_names = list(in_names)
        self.out_names = out_names
        self.out_avals = out_avals
        self.zero_outs = zero_outs
        all_in_names = in_names + out_names + (
            [partition_name] if partition_name else []
        )
        donate = tuple(range(self.n_params, self.n_params + len(out_names)))

        def _body(*args):
            operands = list(args)
            if partition_name is not None:
                operands.append(bass2jax.partition_id_tensor())
            outs = bass2jax._bass_exec_p.bind(
                *operands,
                out_avals=tuple(out_avals),
                in_names=tuple(all_in_names),
                out_names=tuple(out_names),
                lowering_input_output_aliases=(),
                sim_require_finite=True,
                sim_require_nnan=True,
                nc=nc,
            )
            return tuple(outs)

        devices = jax.devices()[:N_CORES]
        self.mesh = Mesh(np.asarray(devices), ("core",))
        in_specs = (PartitionSpec("core"),) * (self.n_params + len(out_names))
        out_specs = (PartitionSpec("core"),) * len(out_names)
        self.sharded = jax.jit(
            shard_map(
                _body,
                mesh=self.mesh,
                in_specs=in_specs,
                out_specs=out_specs,
                check_rep=False,
            ),
            donate_argnums=donate,
            keep_unused=True,
        )

    def concat_inputs(self, in_maps):
        return [
            np.concatenate([np.asarray(m[name]) for m in in_maps], axis=0)
            for name in self.in_names
        ]

    def zeros(self):
        return [
            np.zeros((N_CORES * s[0], *s[1:]), d) for s, d in self.zero_outs
        ]

    def run(self, concat_in, zeros):
        out_arrs = self.sharded(*concat_in, *zeros)
        return out_arrs

    def split(self, out_arrs):
        res = []
        for c in range(N_CORES):
            res.append(
                {
                    name: np.asarray(out_arrs[i]).reshape(
                        N_CORES, *self.out_avals[i].shape
                    )[c]
                    for i, name in enumerate(self.out_names)
                }
            )
        return res


def _get_runner():
    if "runner" not in _CACHE:
        _CACHE["runner"] = _Runner(_get_program())
    return _CACHE["runner"]


def _shard_inputs(X, Wq, bq, Wk, bk, Wv, bv, Wo, bo):
    import ml_dtypes

    bf16 = ml_dtypes.bfloat16
    in_maps = []
    for c in range(N_CORES):
        b, hg = divmod(c, HG)
        cols = slice(HD * hg, HD * (hg + 1))
        sq = 2.0 ** -1.5  # split 1/sqrt(D)=1/8 over Q and K for fp8 range
        bqk = np.stack(
            [
                bq[cols][:P] * sq,
                bq[cols][P:] * sq,
                bk[cols][:P] * sq,
                bk[cols][P:] * sq,
            ],
            axis=1,
        ).astype(np.float32)
        f8 = ml_dtypes.float8_e4m3
        xt = np.ascontiguousarray(X[b].T)
        in_maps.append(
            {
                "XT": xt.astype(bf16),
                "XT8": xt.astype(f8),
                "WQK8": np.concatenate(
                    [Wq[:, cols] * (sq * 64), Wk[:, cols] * (sq * 64)], axis=1
                ).astype(f8),
                "WV": Wv[:, cols].astype(bf16),
                "BQK": bqk,
                "BV": bv[cols].reshape(1, HD).astype(bf16),
                "WO": np.ascontiguousarray(Wo[cols, :]).astype(bf16),
            }
        )
    return in_maps


def kernel(X, Wq, bq, Wk, bk, Wv, bv, Wo, bo):
    X = np.asarray(X, dtype=np.float32)
    Wq, bq = np.asarray(Wq, np.float32), np.asarray(bq, np.float32)
    Wk, bk = np.asarray(Wk, np.float32), np.asarray(bk, np.float32)
    Wv, bv = np.asarray(Wv, np.float32), np.asarray(bv, np.float32)
    Wo, bo = np.asarray(Wo, np.float32), np.asarray(bo, np.float32)

    runner = _get_runner()
    in_maps = _shard_inputs(X, Wq, bq, Wk, bk, Wv, bv, Wo, bo)
    res = runner.split(runner.run(runner.concat_inputs(in_maps), runner.zeros()))

    out = np.empty((B, T, C), dtype=np.float32)
    for b in range(B):
        acc = np.zeros((T, C), dtype=np.float64)
        for hg in range(HG):
            acc += res[HG * b + hg]["OUT"].astype(np.float64)
        out[b] = (acc + bo.astype(np.float64)).astype(np.float32)
    return out


# revision 60
# speedup vs baseline: 1.0395x; 1.0105x over previous
"""Causal multi-head attention block (B=2, T=2048, C=1024, H=16) on 8 TRN2 cores.

Sharding: tensor-parallel over heads x data-parallel over batch.
Core c handles batch b = c // 4 and head-group hg = c % 4 (4 heads = 256 of
the 1024 channel columns). Each core computes, for its batch and heads:
    QT/KT = (Wqk/8^0.5-ish)^T X^T + b  (fp8e4m3 DoubleRow matmuls; host sends
            fp8 copies of X^T and 64x-scaled Wq|Wk, rescaled 2^-6 in the
            fused bias step; 1/sqrt(D) split over Q and K for fp8 range)
    V     = X Wv + bv            (bf16 matmuls, bf16 V + ones column)
    S^T   = K Q^T per 128-key chunk (fp8 DoubleRow, zero-padded 2nd k-tile)
    P^T   = exp(S^T) -> bf16; causal mask applied post-exp as a
            multiplicative 0/1 [128,128] block on the Pool engine
    O     = P V per 128-query chunk (bf16): out[q,0:64]=sum(P*V),
            out[q,64]=sum(P); per-partition softmax normalize
    O^T   via PE transpose (identity matmul) of the [q, 2*64] head-pair block
    partial = O^T rows @ Wo_rows_slice -> OUT bf16 [2048, 1024]
Host sums the 4 partials per batch and adds bo.

Schedule notes (engines execute their streams IN ORDER; emission = schedule):
 - all input DMAs ride the SP queue (ACT-queue DMAs would serialize ahead of
   the exps in the ACT sequencer stream)
 - each head's AV+normalize block is deferred past the next head's first two
   score pairs so ACT streams exps continuously
 - stage-A/C units are interleaved as PE fillers inside the attention streams
 - matmul PSUM outputs must be bank-aligned; transposes need their own
   PSUM tag (sharing the po slot corrupts under rotation)
"""

from contextlib import ExitStack

import numpy as np

import concourse.bacc as bacc
import concourse.mybir as mybir
import concourse.tile as tile
from concourse.bass_utils import run_bass_kernel_spmd

B, T, C, H, D = 2, 2048, 1024, 16, 64
N_CORES = 8
HG = 4                  # head-groups (tensor parallel)
HPC = H // HG           # heads per core = 4
HD = HPC * D            # channel slice per core = 256
P = 128                 # partitions
NT = T // 512           # 4 i-tiles of 512
NIC = T // P            # 16 i-chunks of 128
NKC = C // P            # 8 contraction chunks of 128
F32 = mybir.dt.float32
FP8 = mybir.dt.float8e4
DR = mybir.MatmulPerfMode.DoubleRow
AF = mybir.ActivationFunctionType

MM_DT = mybir.dt.bfloat16   # projection/output matmul dtype
N_EX = 26                   # exp-pair buffers (cross-tile AV deferral liveness)
DEFER_AV = True             # emit each head's AV block after next head's scores

_CACHE: dict = {}


def _build_program():
    import ml_dtypes

    bf16 = ml_dtypes.bfloat16
    nc = bacc.Bacc("TRN2", debug=False)

    XT = nc.dram_tensor("XT", [C, T], MM_DT, kind="ExternalInput").ap()
    XT8 = nc.dram_tensor("XT8", [C, T], FP8, kind="ExternalInput").ap()
    WQK8 = nc.dram_tensor("WQK8", [C, 2 * HD], FP8, kind="ExternalInput").ap()
    WV = nc.dram_tensor("WV", [C, HD], MM_DT, kind="ExternalInput").ap()
    BQK = nc.dram_tensor("BQK", [P, 4], F32, kind="ExternalInput").ap()
    BV = nc.dram_tensor("BV", [1, HD], MM_DT, kind="ExternalInput").ap()
    WO = nc.dram_tensor("WO", [HD, C], MM_DT, kind="ExternalInput").ap()
    OUT = nc.dram_tensor("OUT", [T, C], MM_DT, kind="ExternalOutput").ap()


    # Multiplicative causal mask for the diagonal 128x128 block of each
    # diagonal key-chunk: element (p, j) valid iff j >= p.
    m01 = (np.arange(128)[None, :] >= np.arange(128)[:, None]).astype(bf16)
    CPK = nc.inline_tensor(
        np.concatenate([m01, np.eye(128, dtype=bf16)], axis=1), name="cpk"
    ).ap()
    ONES = nc.inline_tensor(np.ones((1, P), bf16), name="ones_c").ap()
    VONES = nc.inline_tensor(np.ones((P, NIC * HPC), bf16), name="vones_c").ap()

    with tile.TileContext(nc) as tc:
        _trace_kernel(tc, XT, XT8, WQK8, WV, BQK, BV, WO, OUT, CPK, ONES, VONES)
    nc.compile()
    return nc


def _trace_kernel(tc, XT, XT8, WQK8, WV, BQK, BV, WO, OUT, CPK, ONES, VONES):
    nc = tc.nc

    with ExitStack() as ctx:
        consts = ctx.enter_context(tc.tile_pool(name="consts", bufs=1))
        wpool = ctx.enter_context(tc.tile_pool(name="weights", bufs=1))
        xpool = ctx.enter_context(tc.tile_pool(name="xt", bufs=1))
        qkv = ctx.enter_context(tc.tile_pool(name="qkv", bufs=1))

        # Two HWDGE queues: SP (nc.sync) and Activation (nc.scalar).
        qs, qa = nc.sync, nc.scalar

        # ---- tiles ----
        cpk_sb = consts.tile([P, 2 * P], MM_DT, name="cpk_sb")
        m01_sb = cpk_sb[:, 0:P]
        idn_sb = cpk_sb[:, P : 2 * P]
        ones_sb = consts.tile([1, P], MM_DT, name="ones_sb")
        bias_sb = consts.tile([P, 4], F32, name="bias_sb")  # bq m0,m1, bk m0,m1
        bv_sb = consts.tile([1, HD], MM_DT, name="bv_sb")
        wv_sb2 = wpool.tile([P, NKC, HD], MM_DT, name="wv_sb2")
        w8_sb = wpool.tile([P, 4, 2, 2 * HD], FP8, name="w8_sb")
        x8_sb = xpool.tile([P, 4, 2, T], FP8, name="x8_sb")
        wo_sb = wpool.tile([P, 2, C], MM_DT, name="wo_sb")
        xts = [
            xpool.tile([P, T], MM_DT, name=f"xt{kc}", tag=f"xt{kc}")
            for kc in range(NKC)
        ]
        # Q^T/K^T fp8 [P, 2, T]: k-tile 0 data, k-tile 1 zeros (DoubleRow pad)
        qt_sb = [qkv.tile([P, 2, T], FP8, name=f"qt{m}", tag=f"qt{m}") for m in range(2)]
        kt_sb = [qkv.tile([P, 2, T], FP8, name=f"kt{m}", tag=f"kt{m}") for m in range(2)]
        # V bf16 [P, ic, head, D+1]: ones in col D
        v_sb = qkv.tile([P, NIC, HPC, D + 1], MM_DT, name="v_sb")
        ot_sb = [qkv.tile([P, T], MM_DT, name=f"ot{m}", tag=f"ot{m}") for m in range(2)]
        # exp pair buffers [P, 1024] bf16 (chunk jj at cols 512*jj for full
        # pairs; diagonal pairs packed contiguously), manually rotated
        exbufs = [
            qkv.tile([P, 1024], MM_DT, name=f"ex{i}", tag=f"ex{i}")
            for i in range(N_EX)
        ]
        ex_idx = [0]

        def next_ex():
            b = exbufs[ex_idx[0] % N_EX]
            ex_idx[0] += 1
            return b

        def wv_c(kc):
            return wv_sb2[:, kc, :]

        # Preload the ACT Exp table while the first DMAs stream.
        scx = consts.tile([1, 1], F32, name="scx")
        nc.vector.memset(scx, 0.0)
        scy = consts.tile([1, 1], F32, name="scy")
        nc.scalar.activation(scy, scx, AF.Exp)

        # ---- loads ----
        # All input DMAs ride the SP queue (its sequencer is otherwise idle;
        # DMAs on the ACT queue would serialize ahead of the first exp).
        qs.dma_start(w8_sb, WQK8.rearrange("(a b p) c -> p a b c", p=P, b=2))
        for kc in range(NKC):
            qs.dma_start(
                x8_sb[:, kc // 2, kc % 2, 0:512],
                XT8[P * kc : P * (kc + 1), 0:512],
            )
        qs.dma_start(bias_sb, BQK)
        qs.dma_start(cpk_sb, CPK)
        qs.dma_start(ones_sb, ONES)
        qs.dma_start(wv_sb2, WV.rearrange("(a p) c -> p a c", p=P))
        for kc in range(NKC):
            qs.dma_start(xts[kc][:, 0:512], XT[kc * P : (kc + 1) * P, 0:512])
        qs.dma_start(bv_sb, BV)
        qs.dma_start(v_sb[:, :, :, D : D + 1], VONES)
        for kc in range(NKC):
            qs.dma_start(
                x8_sb[:, kc // 2, kc % 2, 512:T], XT8[P * kc : P * (kc + 1), 512:T]
            )
        for t in range(1, NT):
            sl_ = slice(512 * t, 512 * (t + 1))
            for kc in range(NKC):
                qs.dma_start(xts[kc][:, sl_], XT[kc * P : (kc + 1) * P, sl_])
        qs.dma_start(wo_sb, WO.rearrange("(a p) c -> p a c", p=P))

        psum = ctx.enter_context(tc.tile_pool(name="psum", bufs=2, space="PSUM"))
        npool = ctx.enter_context(tc.tile_pool(name="npool", bufs=2))
        opool = ctx.enter_context(tc.tile_pool(name="opool", bufs=3))

        # ---- stage A: projections for one i-tile ----
        def stage_a(t):
            sl = slice(512 * t, 512 * (t + 1))
            for m in range(2):
                msl = slice(P * m, P * (m + 1))
                pqk = psum.tile([P, 1024], F32, tag="big", bufs=3)
                for kc in range(NKC):
                    nc.tensor.matmul(
                        pqk[:, 0:512],
                        lhsT=wq_c(kc, msl),
                        rhs=xts[kc][:, sl],
                        start=(kc == 0),
                        stop=(kc == NKC - 1),
                    )
                    nc.tensor.matmul(
                        pqk[:, 512:1024],
                        lhsT=wk_c(kc, msl),
                        rhs=xts[kc][:, sl],
                        start=(kc == 0),
                        stop=(kc == NKC - 1),
                    )
                with nc.allow_low_precision(reason="fp8 scores"):
                    nc.vector.tensor_scalar_add(
                        qt_sb[m][:, 0, sl], pqk[:, 0:512], bias_sb[:, m : m + 1]
                    )
                    nc.vector.tensor_scalar_add(
                        kt_sb[m][:, 0, sl], pqk[:, 512:1024], bias_sb[:, 2 + m : 3 + m]
                    )
            for ic in range(4 * t, 4 * (t + 1)):
                isl = slice(P * ic, P * (ic + 1))
                pv = psum.tile([P, HD], F32, tag="po", bufs=2)
                for kc in range(NKC):
                    nc.tensor.matmul(
                        pv,
                        lhsT=xts[kc][:, isl],
                        rhs=wv_c(kc),
                        start=(kc == 0),
                        stop=False,
                    )
                nc.tensor.matmul(
                    pv, lhsT=ones_sb, rhs=bv_sb, start=False, stop=True
                )
                with nc.allow_low_precision(reason="fp8 AV"):
                    nc.vector.tensor_copy(
                        v_sb[:, ic, :, 0:D], pv.rearrange("p (h d) -> p h d", d=D)
                    )

        # One-time zeroing of the DoubleRow pad k-tiles (emitted after
        # stage_a(0) so the first bias-adds aren't delayed on DVE).
        def pad_zeros():
            nc.vector.memset(qt_sb[0][:, 1, :], 0.0)
            nc.vector.memset(kt_sb[0][:, 1, :], 0.0)
            nc.gpsimd.memset(qt_sb[1][:, 1, :], 0.0)
            nc.gpsimd.memset(kt_sb[1][:, 1, :], 0.0)

        # ---- stage B: attention for one i-tile ----
        # Emission is software-pipelined: each pair-unit emits its scores+exp
        # immediately, its AV matmuls one unit later (so they never jam the
        # 4-deep PE wait queue behind a pending exp), and the per-head-pair
        # normalize/transpose work one more unit later. `fillers` (stage A/C
        # units of other tiles) are interspersed between pair-units so the PE
        # stream has dense independent work during the ACT-bound stretches.
        widths = {0: 512, 1: 384, 2: 256, 3: 128}
        on2_ref = [None, None]
        deferred = [None]   # carried AV+normalize block, may cross tiles
        head_unit = [0]

        def stage_b(t, fillers, per_tick=1):
            sl = slice(512 * t, 512 * (t + 1))
            n_units = (2 * t + 2) * HPC
            fill_every = (
                max(1, (n_units * per_tick) // max(1, len(fillers))) if fillers else 0
            )
            unit_no = [0]
            pending = []  # deferred transpose blocks, emitted one unit later

            head_unit = [0]

            def tick():
                unit_no[0] += 1
                head_unit[0] += 1
                if head_unit[0] == 2 and deferred[0] is not None:
                    deferred[0]()
                    deferred[0] = None
                if pending:
                    pending.pop(0)()
                if unit_no[0] % fill_every == 0 if fillers else False:
                    for _ in range(min(per_tick, len(fillers))):
                        fillers.pop(0)()

            for l in range(HPC):
                mc, ro = l // 2, 64 * (l % 2)
                qrow = slice(ro, ro + 64)
                if l % 2 == 0:
                    on2 = npool.tile(
                        [P, 4, 2, D], MM_DT, name=f"on{mc}", tag=f"on{mc}", bufs=2
                    )
                    on2_ref[mc] = on2
                else:
                    on2 = on2_ref[mc]

                # scores + exp stream; each pair's exp'd chunks stay live in
                # their exbuf until this head's AV accumulations consume them
                exs = []  # exbuf per pair, chunk jj at column 512*jj (full)
                for jcp in range(2 * t):
                    ps = psum.tile([P, 1024], F32, name="ps", tag="big", bufs=3)
                    for half in (0, 1):
                        jc = 2 * jcp + half
                        nc.tensor.matmul(
                            ps[:, 512 * half : 512 * (half + 1)],
                            lhsT=kt_sb[mc][qrow, :, P * jc : P * (jc + 1)],
                            rhs=qt_sb[mc][qrow, :, sl],
                            start=True,
                            stop=True,
                            perf_mode=DR,
                        )
                    exb = next_ex()
                    with nc.allow_low_precision(reason="fp8 AV"):
                        nc.scalar.activation(exb, ps, AF.Exp)
                    exs.append(exb)
                    tick()

                # diagonal chunk pairs: chunk k covers tile-local queries
                # [128k, 512), packed contiguously in PSUM/exbuf
                for ka, kb in ((0, 1), (2, 3)):
                    wa, wb = widths[ka], widths[kb]
                    ps = psum.tile([P, 1024], F32, name="ps", tag="big", bufs=3)
                    for base, k, w in ((0, ka, wa), (wa, kb, wb)):
                        nc.tensor.matmul(
                            ps[:, base : base + w],
                            lhsT=kt_sb[mc][qrow, :, P * (4 * t + k) : P * (4 * t + k + 1)],
                            rhs=qt_sb[mc][qrow, :, 512 * (t + 1) - w : 512 * (t + 1)],
                            start=True,
                            stop=True,
                            perf_mode=DR,
                        )
                    exb = next_ex()
                    with nc.allow_low_precision(reason="fp8 AV"):
                        nc.scalar.activation(
                            exb[:, 0 : wa + wb], ps[:, 0 : wa + wb], AF.Exp
                        )
                    # causal mask on the leading 128x128 triangle blocks
                    # (post-exp multiplicative, on Pool: off the ACT path)
                    nc.gpsimd.tensor_mul(exb[:, 0:128], exb[:, 0:128], m01_sb)
                    nc.gpsimd.tensor_mul(
                        exb[:, wa : wa + 128], exb[:, wa : wa + 128], m01_sb
                    )
                    exs.append(exb)
                    tick()

                # AV per query chunk: serial accumulation into a bank-aligned
                # [P, 65] PSUM tile over chunks jc = 0 .. 4t+qi, then
                # normalize by the softmax sum (column D, per-partition).
                # Deferred one head so ACT streams the next head's exps while
                # the PE drains these accumulations.
                def av_block(l=l, mc=mc, on2=on2, exs=exs):
                    def ex_col(jc):
                        # (exbuf, column base of chunk jc's block)
                        if jc < 4 * t:
                            return exs[jc // 2], 512 * (jc % 2)
                        k = jc - 4 * t
                        if k < 2:
                            return exs[2 * t], 0 if k == 0 else widths[0]
                        return exs[2 * t + 1], 0 if k == 2 else widths[2]

                    for qi in range(4):
                        po_raw = psum.tile([P, 256], F32, name="po_raw", tag="po", bufs=1)
                        po = po_raw[:, 0 : D + 1]
                        for jc in range(4 * t + qi + 1):
                            exb, base = ex_col(jc)
                            k0 = max(0, jc - 4 * t)
                            off = base + 128 * (qi - k0)
                            nc.tensor.matmul(
                                po,
                                lhsT=exb[:, off : off + 128],
                                rhs=v_sb[:, jc, l, :],
                                start=(jc == 0),
                                stop=(jc == 4 * t + qi),
                                skip_group_check=True,
                            )
                        rc = npool.tile([P, 1], F32, name="rc", tag="rc", bufs=2)
                        nc.vector.reciprocal(rc, po[:, D : D + 1])
                        with nc.allow_low_precision(reason="bf16 out"):
                            nc.vector.tensor_scalar_mul(
                                on2[:, qi, l % 2, :], po[:, 0:D], rc
                            )
                    if l % 2 == 1:
                        for qi in range(4):
                            tp = psum.tile([P, P], MM_DT, name="tp", tag="tp", bufs=1)
                            nc.tensor.transpose(tp, on2[:, qi, :, :], idn_sb)
                            csl = slice(P * (4 * t + qi), P * (4 * t + qi + 1))
                            nc.vector.tensor_copy(ot_sb[mc][:, csl], tp)

                if DEFER_AV:
                    if deferred[0] is not None:
                        deferred[0]()
                    deferred[0] = av_block
                    head_unit[0] = 0
                else:
                    av_block()
            while pending:
                pending.pop(0)()
            while fillers:
                fillers.pop(0)()

        # ---- stage C: output projection, one unit per 128-row i-chunk ----
        def stage_c_unit(ic):
            def emit():
                isl = slice(P * ic, P * (ic + 1))
                ob = opool.tile([P, C], MM_DT, name="ob", tag="ob")
                pc = psum.tile([P, 1024], F32, name="pc", tag="big", bufs=3)
                for n in (0, 1):
                    for kc in range(2):
                        nc.tensor.matmul(
                            pc[:, 512 * n : 512 * (n + 1)],
                            lhsT=ot_sb[kc][:, isl],
                            rhs=wo_sb[:, kc, 512 * n : 512 * (n + 1)],
                            start=(kc == 0),
                            stop=(kc == 1),
                        )
                with nc.allow_low_precision(reason="bf16 out"):
                    nc.vector.tensor_copy(ob, pc)
                qs.dma_start(OUT[isl, :], ob)

            return emit

        def stage_a_units(t, which="qkv"):
            units = []
            sl = slice(512 * t, 512 * (t + 1))
            for m in range(2):
                def emit_qk(m=m):
                    msl = slice(P * m, P * (m + 1))
                    kmsl = slice(HD + P * m, HD + P * (m + 1))
                    pqk = psum.tile([P, 1024], F32, name="pqk", tag="big", bufs=3)
                    for kcp in range(4):
                        nc.tensor.matmul(
                            pqk[:, 0:512], lhsT=w8_sb[:, kcp, :, msl],
                            rhs=x8_sb[:, kcp, :, sl],
                            start=(kcp == 0), stop=(kcp == 3), perf_mode=DR,
                        )
                    for kcp in range(4):
                        nc.tensor.matmul(
                            pqk[:, 512:1024], lhsT=w8_sb[:, kcp, :, kmsl],
                            rhs=x8_sb[:, kcp, :, sl],
                            start=(kcp == 0), stop=(kcp == 3), perf_mode=DR,
                        )
                    mul, add = mybir.AluOpType.mult, mybir.AluOpType.add
                    with nc.allow_low_precision(reason="fp8 scores"):
                        nc.vector.tensor_scalar(
                            qt_sb[m][:, 0, sl], pqk[:, 0:512], 2.0 ** -6,
                            bias_sb[:, m : m + 1], mul, add,
                        )
                        nc.vector.tensor_scalar(
                            kt_sb[m][:, 0, sl], pqk[:, 512:1024], 2.0 ** -6,
                            bias_sb[:, 2 + m : 3 + m], mul, add,
                        )
                if "qk" in which:
                    units.append(emit_qk)
            for ic in range(4 * t, 4 * (t + 1)) if "v" in which else []:
                def emit_v(ic=ic):
                    isl = slice(P * ic, P * (ic + 1))
                    pv = psum.tile([P, HD], F32, name="pv", tag="po", bufs=1)
                    for kc in range(NKC):
                        nc.tensor.matmul(
                            pv, lhsT=xts[kc][:, isl], rhs=wv_c(kc),
                            start=(kc == 0), stop=False,
                        )
                    nc.tensor.matmul(pv, lhsT=ones_sb, rhs=bv_sb, start=False, stop=True)
                    with nc.allow_low_precision(reason="fp8 AV"):
                        nc.vector.tensor_copy(
                            v_sb[:, ic, :, 0:D],
                            pv.rearrange("p (h d) -> p h d", d=D),
                        )
                units.append(emit_v)
            return units

        # Emission: pads while the first DMAs stream, a(0), then attention
        # tiles in order 0,2,3,1 with later-stage units interleaved as PE
        # fillers inside the ACT-bound attention streams.
        pad_zeros()
        for u in stage_a_units(0, "qk"):
            u()
        stage_b(
            0,
            stage_a_units(0, "v") + stage_a_units(1) + stage_a_units(2),
            per_tick=2,
        )
        stage_b(2, stage_a_units(3) + [stage_c_unit(ic) for ic in range(0, 4)])
        stage_b(3, [stage_c_unit(ic) for ic in range(8, 12)])
        stage_b(1, [stage_c_unit(ic) for ic in range(12, 16)])
        if deferred[0] is not None:
            deferred[0]()
            deferred[0] = None
        for ic in range(4, 8):
            stage_c_unit(ic)()



def _get_program():
    if "nc" not in _CACHE:
        _CACHE["nc"] = _build_program()
    return _CACHE["nc"]


class _Runner:
    """Reusable SPMD executor (adapted from concourse.bass2jax.run_bass_via_pjrt)
    so repeated kernel() calls reuse one compiled executable."""

    def __init__(self, nc):
        import jax
        import concourse.mybir as mb
        from jax.sharding import Mesh, PartitionSpec
        from jax.experimental.shard_map import shard_map
        from concourse import bass2jax

        bass2jax.install_neuronx_cc_hook()
        self.jax = jax
        self.nc = nc
        partition_name = (
            nc.partition_id_tensor.name if nc.partition_id_tensor else None
        )
        in_names, out_names, out_avals, zero_outs = [], [], [], []
        for alloc in nc.m.functions[0].allocations:
            if not isinstance(alloc, mb.MemoryLocationSet):
                continue
            name = alloc.memorylocations[0].name
            if alloc.kind == "ExternalInput":
                if name != partition_name:
                    in_names.append(name)
            elif alloc.kind == "ExternalOutput":
                shape = tuple(alloc.tensor_shape)
                dtype = mb.dt.np(alloc.dtype)
                out_names.append(name)
                out_avals.append(jax.core.ShapedArray(shape, dtype))
                zero_outs.append((shape, dtype))
        self.n_params = len(in_names)
        self.in_names = list(in_names)
        self.out_names = out_names
        self.out_avals = out_avals
        self.zero_outs = zero_outs
        all_in_names = in_names + out_names + (
            [partition_name] if partition_name else []
        )
        donate = tuple(range(self.n_params, self.n_params + len(out_names)))

        def _body(*args):
            operands = list(args)
            if partition_name is not None:
                operands.append(bass2jax.partition_id_tensor())
            outs = bass2jax._bass_exec_p.bind(
                *operands,
                out_avals=tuple(out_avals),
                in_names=tuple(all_in_names),
                out_names=tuple(out_names),
                lowering_input_output_aliases=(),
                sim_require_finite=True,
                sim_require_nnan=True,
                nc=nc,
            )
            return tuple(outs)

        devices = jax.devices()[:N_CORES]
        self.mesh = Mesh(np.asarray(devices), ("core",))
        in_specs = (PartitionSpec("core"),) * (self.n_params + len(out_names))
        out_specs = (PartitionSpec("core"),) * len(out_names)
        self.sharded = jax.jit(
            shard_map(
                _body,
                mesh=self.mesh,
                in_specs=in_specs,
                out_specs=out_specs,
                check_rep=False,
            ),
            donate_argnums=donate,
            keep_unused=True,
        )

    def concat_inputs(self, in_maps):
        return [
            np.concatenate([np.asarray(m[name]) for m in in_maps], axis=0)
            for name in self.in_names
        ]

    def zeros(self):
        return [
            np.zeros((N_CORES * s[0], *s[1:]), d) for s, d in self.zero_outs
        ]

    def run(self, concat_in, zeros):
        out_arrs = self.sharded(*concat_in, *zeros)
        return out_arrs

    def split(self, out_arrs):
        res = []
        for c in range(N_CORES):
            res.append(
                {
                    name: np.asarray(out_arrs[i]).reshape(
                        N_CORES, *self.out_avals[i].shape
                    )[c]
                    for i, name in enumerate(self.out_names)
                }
            )
        return res


def _get_runner():
    if "runner" not in _CACHE:
        _CACHE["runner"] = _Runner(_get_program())
    return _CACHE["runner"]


def _shard_inputs(X, Wq, bq, Wk, bk, Wv, bv, Wo, bo):
    import ml_dtypes

    bf16 = ml_dtypes.bfloat16
    in_maps = []
    for c in range(N_CORES):
        b, hg = divmod(c, HG)
        cols = slice(HD * hg, HD * (hg + 1))
        sq = 2.0 ** -1.5  # split 1/sqrt(D)=1/8 over Q and K for fp8 range
        bqk = np.stack(
            [
                bq[cols][:P] * sq,
                bq[cols][P:] * sq,
                bk[cols][:P] * sq,
                bk[cols][P:] * sq,
            ],
            axis=1,
        ).astype(np.float32)
        f8 = ml_dtypes.float8_e4m3
        xt = np.ascontiguousarray(X[b].T)
        in_maps.append(
            {
                "XT": xt.astype(bf16),
                "XT8": xt.astype(f8),
                "WQK8": np.concatenate(
                    [Wq[:, cols] * (sq * 64), Wk[:, cols] * (sq * 64)], axis=1
                ).astype(f8),
                "WV": Wv[:, cols].astype(bf16),
                "BQK": bqk,
                "BV": bv[cols].reshape(1, HD).astype(bf16),
                "WO": np.ascontiguousarray(Wo[cols, :]).astype(bf16),
            }
        )
    return in_maps


def kernel(X, Wq, bq, Wk, bk, Wv, bv, Wo, bo):
    X = np.asarray(X, dtype=np.float32)
    Wq, bq = np.asarray(Wq, np.float32), np.asarray(bq, np.float32)
    Wk, bk = np.asarray(Wk, np.float32), np.asarray(bk, np.float32)
    Wv, bv = np.asarray(Wv, np.float32), np.asarray(bv, np.float32)
    Wo, bo = np.asarray(Wo, np.float32), np.asarray(bo, np.float32)

    runner = _get_runner()
    in_maps = _shard_inputs(X, Wq, bq, Wk, bk, Wv, bv, Wo, bo)
    res = runner.split(runner.run(runner.concat_inputs(in_maps), runner.zeros()))

    out = np.empty((B, T, C), dtype=np.float32)
    for b in range(B):
        acc = np.zeros((T, C), dtype=np.float64)
        for hg in range(HG):
            acc += res[HG * b + hg]["OUT"].astype(np.float64)
        out[b] = (acc + bo.astype(np.float64)).astype(np.float32)
    return out
